# revision 35
# baseline (speedup 1.0000x reference)
"""Trainium2 Bass kernel for nn_CBlock2 (sparse cluster attention block).

Strategy: data-parallel over batch B=8 across 8 cores. Per core, tokens are
host-sorted by cluster id so same-cluster attention pairs lie within a band
of halfwidth EW (32 or 64) around the diagonal. All large GEMMs (QKV, proj,
MLP) run in fp8e4m3 with the DoubleRow perf mode (two 128-deep k-tiles per
instruction); the cluster mask is folded into the score matmul as a second
DoubleRow k-tile of one-hot rows so exp(s-48)==0 for cross-cluster pairs.
Activations are transposed to feature-major via the DMA XBAR (16x128 tile
transpose) using an fp8-pair bitcast; LayerNorm gains fold into the adjacent
weights on the host. Residual stream stays f32.
"""
import sys

sys.path.insert(0, "/opt/trn_rl_repo")

import numpy as np
import ml_dtypes

import concourse.bass as bass
import concourse.mybir as mybir
import concourse.tile as tile
from concourse.bass_utils import run_bass_kernel_spmd

NF8 = ml_dtypes.float8_e4m3
NBF = ml_dtypes.bfloat16

B, N, C, H, PD, CLN = 8, 1024, 512, 8, 256, 64
HD = C // H          # 64
HID = 4 * C          # 2048
LN_EPS = 1e-5
ATT_EPS = 1e-6
P = 128
NT = N // P          # 8 token tiles
FH = HID // P        # 16 hidden tiles
SCALE = HD ** -0.5   # 0.125
ALPHA_Q = 16.0
ALPHA_K = 24.0
BIG = ALPHA_Q * ALPHA_K * SCALE  # 48.0

F32 = mybir.dt.float32
BF = mybir.dt.bfloat16
F8 = mybir.dt.float8e4
DR = mybir.MatmulPerfMode.DoubleRow


def _split_excess_waits(nc, max_waits=1):
    """walrus in this env rejects >1 sync-wait on one instruction; hoist
    excess waits onto same-engine no-op carriers inserted just before."""
    for f in nc.m.functions:
        for bb in f.blocks:
            new_insts = []
            for inst in bb.instructions:
                si = inst.sync_info
                if si is not None and si.on_wait and len(si.on_wait) > max_waits:
                    waits = list(si.on_wait)
                    excess, keep = waits[:-max_waits], waits[-max_waits:]
                    for ci in range(0, len(excess), max_waits):
                        chunk = excess[ci : ci + max_waits]
                        new_insts.append(
                            mybir.InstNoOp(
                                name=f"{inst.name}-ws{ci}",
                                engine=inst.engine,
                                ins=[],
                                outs=[],
                                sync_info=mybir.SyncInfo(on_wait=chunk, on_update=[]),
                            )
                        )
                    inst.sync_info = mybir.SyncInfo(
                        on_wait=keep, on_update=list(si.on_update)
                    )
                new_insts.append(inst)
            bb.instructions = new_insts


def _band(jt, ew):
    i0 = max(0, jt * P - ew)
    i1 = min(N, (jt + 1) * P + ew)
    return i0, i1


def _qoffs(ew):
    """per-half psum column offsets for j-tiles; 2 j-tiles per 2KB bank."""
    qoff = {}
    for jt in range(NT):
        s = jt % 4
        if s == 0:
            qoff[jt] = 0
        elif s == 1:
            qoff[jt] = _band(jt - 1, ew)[1] - _band(jt - 1, ew)[0]
        elif s == 2:
            qoff[jt] = 512
        else:
            qoff[jt] = 512 + _band(jt - 1, ew)[1] - _band(jt - 1, ew)[0]
    return qoff


def _build_program(ew: int, mlp_chunks: int = 2):
    nc = bass.Bass()

    x_d = nc.declare_dram_parameter("x", [N, C], BF, isOutput=False)
    wq_d = nc.declare_dram_parameter("wqp", [P, 2048], F8, isOutput=False)
    wk_d = nc.declare_dram_parameter("wkp", [P, 2048], F8, isOutput=False)
    wv_d = nc.declare_dram_parameter("wvp", [P, 2048], F8, isOutput=False)
    wp_d = nc.declare_dram_parameter("wpp", [P, 2048], F8, isOutput=False)
    w1_d = nc.declare_dram_parameter("w1p", [P, 8192], F8, isOutput=False)
    w2_d = nc.declare_dram_parameter("w2p", [P, 8192], F8, isOutput=False)
    m1b_d = nc.declare_dram_parameter("m1b", [P, FH], F32, isOutput=False)
    qm_d = nc.declare_dram_parameter("qmsk", [P, N], F8, isOutput=False)
    km_d = nc.declare_dram_parameter("kmsk", [P, N], F8, isOutput=False)
    y_d = nc.declare_dram_parameter("y", [N, C], F32, isOutput=True)

    qoff = _qoffs(ew)
    # exp width per half: offset of last jt + its band width
    expw = {hf: qoff[4 * hf + 3]
            + _band(4 * hf + 3, ew)[1] - _band(4 * hf + 3, ew)[0]
            for hf in range(2)}

    with tile.TileContext(nc) as tc:
        from contextlib import ExitStack

        with ExitStack() as ctx:
            ec = ctx.enter_context
            persist = ec(tc.tile_pool(name="persist", bufs=1))
            ln_pool = ec(tc.tile_pool(name="ln", bufs=4))
            e_pool = ec(tc.tile_pool(name="epool", bufs=18))
            pos_pool = ec(tc.tile_pool(name="pos", bufs=4))
            r_pool = ec(tc.tile_pool(name="rpool", bufs=4))
            y_pool = ec(tc.tile_pool(name="ypool", bufs=2))

            # ---- persistent tiles ----
            X = persist.tile([P, NT, C], BF, tag="X")
            X1 = persist.tile([P, NT, C], F32, tag="X1")
            U = [persist.tile([P, 4, C], F8, tag=f"U{i}", name=f"U{i}")
                 for i in range(2)]
            UT = [persist.tile([P, 8, 2 * P], F8, tag=f"UT{i}", name=f"UT{i}")
                  for i in range(2)]
            # per co-slot: [feats, mask] so the DR k-tile stride is always N
            qA = persist.tile([P, 4, 2, N], F8, tag="qA")
            kA = persist.tile([P, 4, 2, N], F8, tag="kA")
            vext = persist.tile([P, NT, H, HD + 1], BF, tag="vext")
            O8 = [persist.tile([P, 4, C], F8, tag=f"O8{i}", name=f"O8{i}")
                  for i in range(2)]
            OT = [persist.tile([P, 8, 2 * P], F8, tag=f"OT{i}", name=f"OT{i}")
                  for i in range(2)]
            H8 = [persist.tile([P, FH, 512], F8, tag=f"H8{i}", name=f"H8{i}")
                  for i in range(2)]
            WQ = persist.tile([P, 2, 2, C], F8, tag="WQ")
            WK = persist.tile([P, 2, 2, C], F8, tag="WK")
            WV = persist.tile([P, 2, 2, C], F8, tag="WV")
            WP = persist.tile([P, 2, 2, C], F8, tag="WP")
            W1 = persist.tile([P, 2, 2, HID], F8, tag="W1")
            W2 = persist.tile([P, 8, 2, C], F8, tag="W2")
            m1b_t = persist.tile([P, FH], F32, tag="m1b")
            eps_t = persist.tile([P, 1], F32, tag="eps")
            MV = persist.tile([P, NT, 2], F32, tag="MV")
            STD = persist.tile([P, NT], F32, tag="STD")
            NMB = persist.tile([P, NT], F32, tag="NMB")  # -mu*rstd biases

            nbig_t = persist.tile([P, 1], F32, tag="nbig")
            nc.vector.memset(eps_t[:], LN_EPS)
            nc.vector.memset(nbig_t[:], -BIG)
            # ones columns of vext (col HD of each head), set once
            nc.gpsimd.memset(vext[:, :, :, HD : HD + 1], 1.0)

            # ---- input DMAs ----
            for qtr in range(4):
                nc.sync.dma_start(
                    out=X[:, 2 * qtr : 2 * qtr + 2, :],
                    in_=x_d.rearrange("(t p) c -> p t c", p=P)[
                        :, 2 * qtr : 2 * qtr + 2, :
                    ],
                )
            nc.sync.dma_start(out=WV[:].rearrange("p a b c -> p (a b c)"), in_=wv_d[:])
            # masks replicated into all 4 co slots via stride-0 source dim
            for mk, dstA in ((qm_d, qA), (km_d, kA)):
                map_ = mk[:]
                src_b = bass.AP(
                    tensor=map_.tensor, offset=map_.offset,
                    ap=[map_.ap[0], [0, 4], map_.ap[1]],
                )
                nc.sync.dma_start(out=dstA[:, :, 1, :], in_=src_b)
            nc.sync.dma_start(out=WQ[:].rearrange("p a b c -> p (a b c)"), in_=wq_d[:])
            nc.sync.dma_start(out=WK[:].rearrange("p a b c -> p (a b c)"), in_=wk_d[:])

            def layernorm_q(src, dst_u, qtr, scope):
                """free-axis LN of 2 token tiles -> fp8. All stats live in
                quarter-local tiles so quarters pipeline without false WAR
                serialization; normalize alternates ACT/Pool."""
                with nc.named_scope(scope):
                    mvq = ln_pool.tile([P, 2, 2], F32, tag="mvq")
                    stdq = ln_pool.tile([P, 2], F32, tag="stdq")
                    nmbq = ln_pool.tile([P, 2], F32, tag="nmbq")
                    for i_ in range(2):
                        st = ln_pool.tile([P, 6], F32, tag="st")
                        nc.vector.bn_stats(
                            out=st[:], in_=src[:, 2 * qtr + i_, :]
                        )
                        nc.vector.bn_aggr(out=mvq[:, i_, :], in_=st[:])
                    nc.scalar.activation(
                        out=stdq[:], in_=mvq[:, :, 1],
                        func=mybir.ActivationFunctionType.Sqrt,
                        bias=eps_t[:], scale=1.0,
                    )
                    nc.vector.reciprocal(out=stdq[:], in_=stdq[:])
                    nc.vector.scalar_tensor_tensor(
                        out=nmbq[:], in0=mvq[:, :, 0], scalar=-1.0,
                        in1=stdq[:], op0=mybir.AluOpType.mult,
                        op1=mybir.AluOpType.mult,
                    )
                    for i_ in range(2):
                        it = 2 * qtr + i_
                        du = dst_u[it // 4][:, it % 4, :]
                        if i_ == 0:
                            nc.scalar.activation(
                                out=du, in_=src[:, it, :],
                                func=mybir.ActivationFunctionType.Identity,
                                bias=nmbq[:, i_ : i_ + 1],
                                scale=stdq[:, i_ : i_ + 1],
                            )
                        else:
                            nc.gpsimd.tensor_scalar(
                                out=du, in0=src[:, it, :],
                                scalar1=mvq[:, i_, 0:1],
                                scalar2=stdq[:, i_ : i_ + 1],
                                op0=mybir.AluOpType.subtract,
                                op1=mybir.AluOpType.mult,
                            )

            def xbar(src, dst, half):
                """pair-transpose src-half fp8 [P, 4 token tiles, C] into
                dst-half [P, 8, 2P] fp8 (bf16-pair view)."""
                nc.sync.dma_start_transpose(
                    out=dst[half][:].bitcast(BF),
                    in_=src[half][:].bitcast(BF),
                )

            # ---- LN1 + transpose, quarter-pipelined ----
            layernorm_q(X, U, 0, "ln1")
            layernorm_q(X, U, 1, "ln1")
            layernorm_q(X, U, 2, "ln1")
            layernorm_q(X, U, 3, "ln1")
            xbar(U, UT, 0)
            xbar(U, UT, 1)
            # late-phase weight loads: held back on the in-order SP queue by
            # the xbars' data waits, keeping the early DMA window clear
            nc.sync.dma_start(
                out=WP[:].rearrange("p a b c -> p (a b c)"), in_=wp_d[:]
            )
            nc.sync.dma_start(
                out=W1[:].rearrange("p a b c -> p (a b c)"), in_=w1_d[:]
            )
            nc.sync.dma_start(out=m1b_t[:], in_=m1b_d[:])
            nc.sync.dma_start(
                out=W2[:].rearrange("p a b c -> p (a b c)"), in_=w2_d[:]
            )


            def ut_rhs(it, jc):
                """UT slice as DR rhs/lhsT [P, 2 (s), P (tokens)]."""
                g = 2 * (it % 4) + jc
                return UT[it // 4][:, g, :].rearrange("p (n s) -> p s n", s=2)

            # ---- QKV + attention, software-pipelined ----
            # v first (own psum pool, closed before scores start); then per
            # co-slot k/q matmuls+copies interleaved with per-head scores,
            # with AV(h-1) emitted after scores(h) so the PE never stalls on
            # an exp round-trip.
            with nc.named_scope("qkv"), \
                    tc.tile_pool(name="ps_v", bufs=2, space="PSUM") as ps_vp:
                for it in range(NT):
                    ps = ps_vp.tile([P, C], F32, tag="v")
                    for jc in range(2):
                        for s in range(2):
                            for cc in range(2):
                                nc.tensor.matmul(
                                    ps[:, cc * 256 : (cc + 1) * 256],
                                    ut_rhs(it, jc)[:, s, :],
                                    WV[:, jc, s, cc * 256 : (cc + 1) * 256],
                                    start=(jc == 0 and s == 0 and cc == 0),
                                    stop=(jc == 1 and s == 1 and cc == 1),
                                )
                    vdst = vext[:, it, :, 0:HD]
                    vsrc = ps[:].rearrange("p (h d) -> p h d", h=H)
                    if it % 2 == 0:
                        nc.vector.tensor_copy(out=vdst, in_=vsrc)
                    else:
                        nc.scalar.copy(out=vdst, in_=vsrc)

            with nc.named_scope("attn"), \
                    tc.tile_pool(name="ps_qk", bufs=2, space="PSUM") as ps_qk, \
                    tc.tile_pool(name="ps_sc", bufs=2, space="PSUM") as ps_sc, \
                    tc.tile_pool(name="ps_po", bufs=2, space="PSUM") as ps_po:
                ets = {}

                def emit_co_half(co, th):
                    """k then q matmuls+copy for tokens [512*th, 512*(th+1))."""
                    for wt, dstA, eng in (
                        (WK, kA, nc.scalar if co < 2 else None),
                        (WQ, qA, None),
                    ):
                        ps = ps_qk.tile([P, C], F32, tag="qk")
                        for i_, it in enumerate(range(4 * th, 4 * th + 4)):
                            for jc in range(2):
                                nc.tensor.matmul(
                                    ps[:, i_ * P : (i_ + 1) * P],
                                    wt[:, jc, :, co * P : (co + 1) * P],
                                    ut_rhs(it, jc),
                                    start=(i_ == 0 and jc == 0),
                                    stop=(i_ == 3 and jc == 1),
                                    perf_mode=DR,
                                )
                        dst = dstA[:, co, 0, 512 * th : 512 * (th + 1)]
                        if eng is nc.scalar:
                            nc.scalar.copy(out=dst, in_=ps[:])
                        else:
                            nc.vector.tensor_copy(out=dst, in_=ps[:])

                def emit_co(co):
                    emit_co_half(co, 0)
                    emit_co_half(co, 1)

                def emit_scores(h):
                    par, co = h % 2, h // 2
                    pr = slice(64 * par, 64 * par + 64)
                    ets[h] = {}
                    for hf in range(2):
                        ps = ps_sc.tile([P, N], F32, tag="sc")
                        for s4 in range(4):
                            jt = 4 * hf + s4
                            i0, i1 = _band(jt, ew)
                            nc.tensor.matmul(
                                ps[:, qoff[jt] : qoff[jt] + (i1 - i0)],
                                kA[pr, co, :, jt * P : (jt + 1) * P],
                                qA[pr, co, :, i0:i1],
                                start=(s4 % 2 == 0), stop=(s4 % 2 == 1),
                                perf_mode=DR,
                            )
                        et = e_pool.tile([P, N], BF, tag="et")
                        nc.scalar.activation(
                            out=et[:, 0 : expw[hf]], in_=ps[:, 0 : expw[hf]],
                            func=mybir.ActivationFunctionType.Exp,
                            bias=nbig_t[:], scale=float(SCALE),
                        )
                        ets[h][hf] = et

                def emit_av(h, g):
                    if True:
                        po = ps_po.tile([P, 4, HD + 1], F32, tag="po")
                        nmm = []
                        for s4 in range(4):
                            it = 4 * g + s4
                            i0c = _band(it, ew)[0]
                            pieces = [(it, qoff[it] + it * P - i0c, 0, P)]
                            if it > 0:
                                jt = it - 1
                                off = qoff[jt] + it * P - _band(jt, ew)[0]
                                pieces.append((jt, off, 0, ew))
                            if it < NT - 1:
                                jt = it + 1
                                pieces.append((jt, qoff[jt], P - ew, ew))
                            for jt, off, pb, w in pieces:
                                nmm.append((s4, jt, off, pb, w))
                        for n_, (s4, jt, off, pb, w) in enumerate(nmm):
                            nc.tensor.matmul(
                                po[pb : pb + w, s4, :],
                                ets[h][jt // 4][:, off : off + w],
                                vext[:, jt, h, :],
                                start=(n_ == 0), stop=(n_ == len(nmm) - 1),
                                tile_position=(0, pb),
                            )
                        r = r_pool.tile([P, 4], F32, tag="r")
                        nc.vector.reciprocal(r[:], po[:, :, HD])
                        pos = pos_pool.tile([P, 4, HD + 1], F32, tag="pos")
                        nc.vector.tensor_copy(out=pos[:], in_=po[:])
                        for s4 in range(4):
                            it = 4 * g + s4
                            nc.gpsimd.tensor_scalar_mul(
                                O8[g][:, s4, h * HD : (h + 1) * HD],
                                pos[:, s4, 0:HD],
                                r[:, s4 : s4 + 1],
                            )

                emit_co(0)
                emit_scores(0)
                emit_scores(1)
                emit_co_half(1, 0)
                emit_av(0, 0)
                emit_co_half(1, 1)
                emit_scores(2)
                emit_av(1, 0)
                emit_scores(3)
                emit_co_half(2, 0)
                emit_av(2, 0)
                emit_co_half(2, 1)
                emit_scores(4)
                emit_av(3, 0)
                emit_scores(5)
                emit_co_half(3, 0)
                emit_av(4, 0)
                emit_co_half(3, 1)
                emit_scores(6)
                emit_av(5, 0)
                emit_scores(7)
                emit_av(6, 0)
                emit_av(7, 0)
                # first half of O complete: kick its transpose, then run the
                # second-half AVs while proj/LN2 consume OT half 0
                xbar(O8, OT, 0)
                for h2 in range(H):
                    emit_av(h2, 1)

            # ---- O transpose + proj + residual, half-pipelined with LN2 ----
            with nc.named_scope("proj"), \
                    tc.tile_pool(name="ps_pr", bufs=3, space="PSUM") as ps_pr:

                def ot_lhs(it, jc):
                    g = 2 * (it % 4) + jc
                    return OT[it // 4][:, g, :].rearrange("p (n s) -> p s n", s=2)

                def proj_half(half):
                    for it in range(4 * half, 4 * half + 4):
                        ps = ps_pr.tile([P, C], F32, tag="pr")
                        for jc in range(2):
                            for s in range(2):
                                for cc in range(2):
                                    nc.tensor.matmul(
                                        ps[:, cc * 256 : (cc + 1) * 256],
                                        ot_lhs(it, jc)[:, s, :],
                                        WP[:, jc, s, cc * 256 : (cc + 1) * 256],
                                        start=(jc == 0 and s == 0 and cc == 0),
                                        stop=(jc == 1 and s == 1 and cc == 1),
                                    )
                        nc.vector.tensor_add(X1[:, it, :], X[:, it, :], ps[:])

                proj_half(0)
                xbar(O8, OT, 1)
                proj_half(1)

            # ---- LN2 + transpose (reuses U/UT), quarter-pipelined ----
            layernorm_q(X1, U, 0, "ln2")
            layernorm_q(X1, U, 1, "ln2")
            xbar(U, UT, 0)
            layernorm_q(X1, U, 2, "ln2")
            layernorm_q(X1, U, 3, "ln2")
            xbar(U, UT, 1)

            # ---- MLP, token-chunked so MLP2/chunk0 overlaps MLP1/chunk1 ----
            with nc.named_scope("mlp"), \
                    tc.tile_pool(name="ps_m1", bufs=2, space="PSUM") as ps_m1, \
                    tc.tile_pool(name="ps_m2", bufs=2, space="PSUM") as ps_m2:
                itc = NT // mlp_chunks

                def mlp1(chunk):
                    for fh in range(FH):
                        ps = ps_m1.tile([P, itc * P], F32, tag="m1")
                        for i_, it in enumerate(
                            range(chunk * itc, (chunk + 1) * itc)
                        ):
                            for jc in range(2):
                                nc.tensor.matmul(
                                    ps[:, i_ * P : (i_ + 1) * P],
                                    W1[:, jc, :, fh * P : (fh + 1) * P],
                                    ut_rhs(it, jc),
                                    start=(i_ % 4 == 0 and jc == 0),
                                    stop=((i_ % 4 == 3 or i_ == itc - 1) and jc == 1),
                                    perf_mode=DR,
                                )
                        nc.scalar.activation(
                            out=H8[chunk][:, fh, :],
                            in_=ps[:],
                            func=mybir.ActivationFunctionType.Gelu,
                            bias=m1b_t[:, fh : fh + 1], scale=1.0,
                        )

                ytile_box = [None]

                def mlp2(chunk):
                    for it in range(chunk * itc, (chunk + 1) * itc):
                        ps = ps_m2.tile([P, C], F32, tag="m2")
                        for m in range(8):
                            for cc in range(2):
                                nc.tensor.matmul(
                                    ps[:, cc * 256 : (cc + 1) * 256],
                                    H8[it // 4][
                                        :, 2 * m : 2 * m + 2,
                                        (it % 4) * P : (it % 4 + 1) * P,
                                    ],
                                    W2[:, m, :, cc * 256 : (cc + 1) * 256],
                                    start=(m == 0 and cc == 0),
                                    stop=(m == 7 and cc == 1),
                                    perf_mode=DR,
                                )
                        slot = it % 2
                        if slot == 0:
                            yt = y_pool.tile([P, 2, C], F32, tag="y")
                            ytile_box[0] = yt
                        else:
                            yt = ytile_box[0]
                        nc.vector.tensor_add(yt[:, slot, :], X1[:, it, :], ps[:])
                        if slot == 1:
                            nc.sync.dma_start(
                                out=y_d.rearrange("(t p) c -> p t c", p=P)[
                                    :, it - 1 : it + 1, :
                                ],
                                in_=yt[:],
                            )

                for chunk in range(mlp_chunks):
                    mlp1(chunk)
                    mlp2(chunk)

    _split_excess_waits(nc)
    return nc


_PROGRAMS = {}


def _get_program(ew, mlp_chunks=2):
    key = (ew, mlp_chunks)
    if key not in _PROGRAMS:
        _PROGRAMS[key] = _build_program(ew, mlp_chunks)
    return _PROGRAMS[key]


def _gelu_exact(x):
    from math import sqrt, erf

    import numpy as _np

    return 0.5 * x * (1.0 + _np.vectorize(erf)(x / sqrt(2.0)))


def _reference_np(x_token, wq, wk, wv, w_proj, b_proj, g1, b1, g2, b2,
                  w1, bb1, w2, bb2, idx):
    """float64 numpy fallback (used only if fast-path preconditions fail)."""
    x = x_token.astype(np.float64)
    out = np.empty_like(x)
    scale = HD ** -0.5
    for b in range(x.shape[0]):
        xb = x[b]
        mu = xb.mean(-1, keepdims=True)
        var = ((xb - mu) ** 2).mean(-1, keepdims=True)
        t = (xb - mu) / np.sqrt(var + LN_EPS) * g1 + b1
        q = (t @ wq.T).reshape(N, H, HD).transpose(1, 0, 2)
        k = (t @ wk.T).reshape(N, H, HD).transpose(1, 0, 2)
        v = (t @ wv.T).reshape(N, H, HD).transpose(1, 0, 2)
        s = np.einsum("hid,hjd->hij", q, k) * scale
        same = idx[b][None, :, None] == idx[b][None, None, :]
        e = np.exp(s) * same
        attn = (e + ATT_EPS / N) / (e.sum(-1, keepdims=True) + ATT_EPS)
        o = np.einsum("hij,hjd->hid", attn, v)
        o = o.transpose(1, 0, 2).reshape(N, C) @ w_proj.T + b_proj
        xr = xb + o
        mu = xr.mean(-1, keepdims=True)
        var = ((xr - mu) ** 2).mean(-1, keepdims=True)
        hh = (xr - mu) / np.sqrt(var + LN_EPS) * g2 + b2
        m = _gelu_exact(hh @ w1.T + bb1) @ w2.T + bb2
        out[b] = xr + m
    return out.astype(np.float32)


def _pack_contract512(w_eff):
    """pack [Cout, 512] weight for XBAR'd DR contraction: out [128, 2*2*Cout]
    with layout [p, jc, s, m], channel = jc*256 + 2p + s."""
    cout = w_eff.shape[0]
    arr = np.ascontiguousarray(w_eff.T)  # [512 cin, cout]
    return (
        arr.reshape(2, 128, 2, cout).transpose(1, 0, 2, 3).reshape(128, -1)
    ).astype(NF8)


def _pack_w2(w2_eff):
    """pack [C, 2048] for hid-tile-pair DR: [p, m, t, c], hid=(2m+t)*128+p."""
    arr = np.ascontiguousarray(w2_eff.T)  # [2048 hid, C]
    return (
        arr.reshape(8, 2, 128, C).transpose(2, 0, 1, 3).reshape(128, -1)
    ).astype(NF8)


def kernel(**inputs):
    x_token = np.ascontiguousarray(np.asarray(inputs["x_token"], np.float32))
    idx = np.asarray(inputs["idx_cluster"]).astype(np.int64)
    wq = np.asarray(inputs["wq"], np.float32)
    wk = np.asarray(inputs["wk"], np.float32)
    wv = np.asarray(inputs["wv"], np.float32)
    w_proj = np.asarray(inputs["w_proj"], np.float32)
    b_proj = np.asarray(inputs["b_proj"], np.float32)
    g1 = np.asarray(inputs["g1"], np.float32)
    b1 = np.asarray(inputs["b1"], np.float32)
    g2 = np.asarray(inputs["g2"], np.float32)
    b2 = np.asarray(inputs["b2"], np.float32)
    w1 = np.asarray(inputs["w1"], np.float32)
    bb1 = np.asarray(inputs["bb1"], np.float32)
    w2 = np.asarray(inputs["w2"], np.float32)
    bb2 = np.asarray(inputs["bb2"], np.float32)

    ok = idx.min() >= 0 and idx.max() < CLN
    max_cl = 0
    if ok:
        for b in range(B):
            max_cl = max(max_cl, int(np.bincount(idx[b], minlength=CLN).max()))
    # nonzero b1/b_proj/bb2 would need bias paths; setup_inputs() zeroes them
    if (not ok or max_cl > 65 or np.any(b1) or np.any(b_proj) or np.any(bb2)):
        return _reference_np(x_token, wq, wk, wv, w_proj, b_proj, g1, b1,
                             g2, b2, w1, bb1, w2, bb2, idx)
    ew = 32 if max_cl <= 33 else 64

    # fold LN gains into weights; bb1 + w1@b2 becomes the gelu bias
    wqp = _pack_contract512(wq * g1[None, :])
    wkp = _pack_contract512(wk * g1[None, :])
    wvp = _pack_contract512(wv * g1[None, :])
    wpp = _pack_contract512(w_proj)
    w1p = _pack_contract512(w1 * g2[None, :])
    w2p = _pack_w2(w2)
    m1b = (bb1 + w1 @ b2).astype(np.float32).reshape(FH, P).T
    m1b = np.ascontiguousarray(m1b)

    shared = dict(wqp=wqp, wkp=wkp, wvp=wvp, wpp=wpp, w1p=w1p, w2p=w2p,
                  m1b=m1b)

    perms = []
    in_maps = []
    ar = np.arange(CLN)
    for b in range(B):
        perm = np.argsort(idx[b], kind="stable")
        perms.append(perm)
        cid = idx[b][perm]
        onehot = (cid[None, :] == ar[:, None]).astype(np.float32)
        qm = np.zeros((P, N), np.float32)
        qm[0:64] = ALPHA_Q * onehot
        qm[64:128] = ALPHA_Q * onehot
        km = np.zeros((P, N), np.float32)
        km[0:64] = ALPHA_K * onehot
        km[64:128] = ALPHA_K * onehot
        in_maps.append(
            dict(
                shared,
                x=np.ascontiguousarray(x_token[b][perm]).astype(NBF),
                qmsk=qm.astype(NF8),
                kmsk=km.astype(NF8),
            )
        )

    nc = _get_program(ew)
    res = run_bass_kernel_spmd(nc, in_maps, list(range(B)))
    global LAST_RESULTS, LAST_EW
    LAST_RESULTS = res
    LAST_EW = ew
    out = np.empty((B, N, C), np.float32)
    for b in range(B):
        out[b][perms[b]] = res.results[b]["y"]
    return out


LAST_RESULTS = None
LAST_EW = 32
